# revision 30
# baseline (speedup 1.0000x reference)
"""Trainium2 Bass kernel for a transformer decoder block (self-attn + cross-attn + MLP).

Sharding: 8 cores = 4 batches x 2 sequence-halves; each core computes the full
block for its 512 query tokens (k/v over the full sequence; cross k/v over the
full context). Per-core token rotation puts own tokens first so one uniform
SPMD program serves both halves (causal mask = explicit triangle for own-half
keys + per-core scalar exp-bias for other-half keys).

Pipeline design (vs the 703us baseline):
- LayerNorm never blocks the PE: stats are interleaved ones-matmuls; the
  per-token affine (a,b rows via ACT Rsqrt + one fused scalar_tensor_tensor)
  is applied by DVE while the PE continues with the next matmul stream. The
  PE p-state ramp (0.65/1.2/2.4 GHz, 3us to full clock) makes continuous
  streams worth ~1.3x on their own.
- fp8 e4m3 DoubleRow matmuls (2 contraction rows/cycle) for the k/v/k2/v2
  projections, fc1/fc2, and P*V, with x32 (x64 fc2) weight pre-scaling to
  dodge fp8 subnormals; the descale folds into existing evictions (tensor
  _scalar) or the gelu activation scale. q/so/q2/co + scores stay fp16.
- P*V computed transposed (out [q,hd] per head, full 128x128 PE util) with a
  ones column in v giving the softmax denominator; per-head [128,4] batched
  reciprocal; PE transposes (identity matmul) restore feature-major.
- fp16 residual stream; softmax runs without max-subtraction (logits <= ~3,
  pexp <= e^3 << fp8 max 448; masked entries underflow exp to exactly 0).
"""

import numpy as np
from contextlib import ExitStack

import concourse.bass as bass
import concourse.tile as tile
from concourse import bacc, mybir
from concourse.bass_utils import run_bass_kernel_spmd

F32 = mybir.dt.float32
F16 = mybir.dt.float16
F8 = mybir.dt.float8e4
AFT = mybir.ActivationFunctionType
ALU = mybir.AluOpType
DR = mybir.MatmulPerfMode.DoubleRow

B, L, D = 4, 1024, 1024
MCTX = 1024
NH, HD = 16, 64
HID = 4 * D
EPS = 1e-6
SCALE = HD ** -0.5
Q = 512
P = 128
NEG = -30000.0
WS = 32.0    # fp8 weight pre-scale (projections, fc1)
WS2 = 64.0   # fp8 weight pre-scale (fc2)

_CACHE = {}


def _stats(nc, pp, src, ch):
    """LN stats over features for one 512-token chunk of src [128,8,width] f16.
    Returns (A, B) [128,512] f16 broadcast tiles: LN(x) = x*A + B."""
    pmm, tmp, st, bc = pp["pmm"], pp["tmp"], pp["stats"], pp["bcast"]
    ones, eps_t = pp["ones"], pp["eps"]
    cs = slice(ch * 512, ch * 512 + 512)
    ps_s = pmm.tile([P, 512], F32, tag="mm", name=f"st_s")
    ps_q = pmm.tile([P, 512], F32, tag="mm", name=f"st_q")
    for dt in range(8):
        nc.tensor.matmul(ps_s[0:1, :], ones, src[:, dt, cs],
                         start=(dt == 0), stop=(dt == 7))
        sq = tmp.tile([P, 512], F16, tag="sq", bufs=2)
        nc.vector.tensor_mul(sq, src[:, dt, cs], src[:, dt, cs])
        nc.tensor.matmul(ps_q[0:1, :], ones, sq,
                         start=(dt == 0), stop=(dt == 7))
    m2 = st.tile([1, 512], F32, tag="r32", bufs=3, name="m2")
    nc.scalar.activation(m2, ps_s[0:1, :], AFT.Square)
    v2 = st.tile([1, 512], F32, tag="r32", bufs=3, name="v2")
    nc.vector.scalar_tensor_tensor(v2, m2, -1.0 / D, ps_q[0:1, :],
                                   ALU.mult, ALU.add)
    sd = st.tile([1, 512], F32, tag="r32", bufs=3, name="sd")
    nc.scalar.activation(sd, v2, AFT.Sqrt, bias=eps_t, scale=1.0 / D)
    a = st.tile([1, 512], F32, tag="r32", bufs=3, name="a")
    nc.vector.reciprocal_approx_fast(a, sd)
    bm = st.tile([1, 512], F32, tag="r32", bufs=3, name="bm")
    nc.vector.scalar_tensor_tensor(bm, ps_s[0:1, :], -1.0 / D, a,
                                   ALU.mult, ALU.mult)
    a16 = st.tile([1, 512], F16, tag="r16", bufs=2, name="a16")
    nc.vector.tensor_copy(a16, a)
    b16 = st.tile([1, 512], F16, tag="r16", bufs=2, name="b16")
    nc.vector.tensor_copy(b16, bm)
    A = bc.tile([P, 512], F16, tag="A")
    nc.gpsimd.partition_broadcast(A, a16)
    Bt = bc.tile([P, 512], F16, tag="B")
    nc.gpsimd.partition_broadcast(Bt, b16)
    return A, Bt


def _apply(nc, pp, src, ch, A, Bt, dst16, dst8=None):
    """LN apply: dst[:,dt,cs] = src[:,dt,cs]*A + B. dst16 f16 (may be None),
    dst8 optional f8 twin (cast via tmp ring when dst16 is None)."""
    tmp = pp["tmp"]
    cs = slice(ch * 512, ch * 512 + 512)
    for dt in range(8):
        t1 = tmp.tile([P, 512], F16, tag="ap1")
        nc.vector.tensor_mul(t1, src[:, dt, cs], A)
        if dst16 is not None:
            nc.vector.tensor_add(dst16[:, dt, cs], t1, Bt)
            if dst8 is not None:
                nc.vector.tensor_copy(dst8[:, dt, cs], dst16[:, dt, cs])
        else:
            t2 = tmp.tile([P, 512], F16, tag="ap2", bufs=2)
            nc.vector.tensor_add(t2, t1, Bt)
            nc.vector.tensor_copy(dst8[:, dt, cs], t2)


def _proj16(nc, pp, w_dram, rhs, rhs_cols=None):
    """fp16 projection: yields (ft, psum [128,512]) for 8 f-tiles."""
    wpool, pmm = pp["wpool"], pp["pmm"]
    w_ap = w_dram.ap().rearrange("(dt dp) f -> dp dt f", dp=P)
    rc = rhs_cols if rhs_cols is not None else slice(0, 512)
    for c in range(2):
        wc = wpool.tile([P, 8, 512], F16, tag="w16", bufs=2)
        nc.sync.dma_start(out=wc, in_=w_ap[:, :, c * 512:c * 512 + 512])
        for fs in range(4):
            ft = c * 4 + fs
            ps = pmm.tile([P, 512], F32, tag="mm", name=f"p16_{ft}")
            for dt in range(8):
                nc.tensor.matmul(ps, wc[:, dt, fs * P:fs * P + P],
                                 rhs[:, dt, rc], start=(dt == 0), stop=(dt == 7))
            yield ft, ps


def _attention(nc, pp, kT, vt, qT, saT, m01t, tb_t, fillers=None):
    """Attention with feature-major PV: out [65,512] per head (row 64 = softmax
    denominator via the ones column of vt). kT f16 [128,8,1024], vt f8
    [128,8,16,65], qT f16 [128,8,512] -> saT f16 [128,8,512].
    m01t/tb_t non-None => causal self-attention (rotated layout): score
    matmuls + exp skip the fully-masked prefix of own-half k-tile pairs.
    fillers: callables popped one per head to emit independent PE work into
    the ACT/DVE-bound stretches."""
    pscore, ppv = pp["pscore"], pp["ppv"]
    pexp, tmp, srow, bc = pp["pexp"], pp["tmp"], pp["srow"], pp["bcast"]
    masked = m01t is not None
    for h in range(NH):
        ft, fo = h // 2, (h % 2) * HD
        pe_h = pexp.tile([P, 8, 512], F8, tag="pexp", name=f"pe_{h}")
        if masked:
            nc.vector.memset(pe_h[:, 2:4, 0:256], 0.0)
        for t in range(4):
            # live q-range union of k-tile pair (2t, 2t+1) under causal mask
            q0 = 256 * t if (masked and t < 2) else 0
            ps2 = pscore.tile([P, 2, 512], F32, tag="sc", name=f"sc_{h}_{t}")
            for i in range(2):
                kt = 2 * t + i
                nc.tensor.matmul(
                    ps2[:, i, q0:512],
                    kT[fo:fo + HD, ft, kt * P:kt * P + P],
                    qT[fo:fo + HD, ft, q0:512], start=True, stop=True)
            if masked and t < 2:
                et = tmp.tile([P, 2, 512], F16, tag="et", bufs=2)
                nc.scalar.activation(et[:, :, q0:], ps2[:, :, q0:], AFT.Exp)
                nc.gpsimd.tensor_mul(pe_h[:, 2 * t:2 * t + 2, q0:],
                                     et[:, :, q0:],
                                     m01t[:, 2 * t:2 * t + 2, q0:])
            elif masked:
                nc.scalar.activation(pe_h[:, 2 * t:2 * t + 2, :], ps2,
                                     AFT.Exp, bias=tb_t)
            else:
                nc.scalar.activation(pe_h[:, 2 * t:2 * t + 2, :], ps2,
                                     AFT.Exp)
        pvp = ppv.tile([P, 512], F32, tag="pv", name=f"pv_{h}")
        for t in range(4):
            nc.tensor.matmul(pvp[0:HD + 1, :], vt[:, 2 * t:2 * t + 2, h, :],
                             pe_h[:, 2 * t:2 * t + 2, :],
                             start=(t == 0), stop=(t == 3), perf_mode=DR)
        dn = srow.tile([1, 512], F32, tag="dn", bufs=1, name=f"dn_{h}")
        nc.vector.tensor_copy(dn, pvp[HD:HD + 1, :])
        rb = srow.tile([1, 512], F32, tag="rb", bufs=1, name=f"rb_{h}")
        nc.vector.reciprocal_approx_fast(rb, dn)
        rb16 = srow.tile([1, 512], F16, tag="rb16", bufs=1, name=f"rb16_{h}")
        nc.vector.tensor_copy(rb16, rb)
        rbc = bc.tile([HD, 512], F16, tag="rbc", name=f"rbc_{h}")
        nc.gpsimd.partition_broadcast(rbc, rb16)
        nc.vector.tensor_mul(saT[fo:fo + HD, ft, :], pvp[0:HD, :], rbc)
        if fillers:
            fillers.pop(0)()


def build_program():
    nc = bacc.Bacc("TRN2", target_bir_lowering=False, debug=False,
                   enable_asserts=False)

    din = lambda n, shape, dt_=F16: nc.declare_dram_parameter(
        n, shape, dt_, isOutput=False)
    x16 = din("x16", [D, L])             # rotated, feature-major
    ctx16 = din("ctx16", [D, MCTX])
    m01 = din("m01", [Q, Q])             # own-half 0/1 causal mask [keys, q]
    tbias = din("tbias", [P, 1], F32)    # 0 (s=1) or -30000 (s=0) tail bias
    WqT, WsoT = din("WqT", [D, D]), din("WsoT", [D, D])
    Wq2T, WcoT = din("Wq2T", [D, D]), din("WcoT", [D, D])
    Wk8, Wv8 = din("Wk8", [D, D], F8), din("Wv8", [D, D], F8)
    Wk28, Wv28 = din("Wk28", [D, D], F8), din("Wv28", [D, D], F8)
    W18, W28 = din("W18", [D, HID], F8), din("W28", [HID, D], F8)
    outT = nc.declare_dram_parameter("outT", [D, Q], F32, isOutput=True)

    es = {}
    with tile.TileContext(nc) as tc, ExitStack() as top:
        def popen(name, side, bufs=1, **kw):
            s = ExitStack()
            es[name] = s
            return s.enter_context(
                tc.tile_pool(name=name, bufs=bufs, side=side, **kw))

        def pclose(name):
            es.pop(name).close()

        const = top.enter_context(tc.tile_pool(name="const", bufs=1))
        wpool = top.enter_context(tc.tile_pool(name="wpool", bufs=3))
        tmp = top.enter_context(tc.tile_pool(name="tmp", bufs=3))
        stats = top.enter_context(tc.tile_pool(name="stats", bufs=4))
        bcast = top.enter_context(tc.tile_pool(name="bcast", bufs=2))
        srow = top.enter_context(tc.tile_pool(name="srow", bufs=2))
        pexp = top.enter_context(tc.tile_pool(name="pexp", bufs=2))
        pmm = top.enter_context(
            tc.tile_pool(name="pmm", bufs=2, space="PSUM"))
        pscore = top.enter_context(
            tc.tile_pool(name="pscore", bufs=2, space="PSUM"))
        ppv = top.enter_context(
            tc.tile_pool(name="ppv", bufs=2, space="PSUM"))

        ones = const.tile([P, 1], F16)
        nc.vector.memset(ones, 1.0)
        eps_t = const.tile([1, 1], F32)
        nc.vector.memset(eps_t, EPS)
        tb_t = const.tile([P, 1], F32)
        nc.sync.dma_start(out=tb_t, in_=tbias[:, :])

        pp = {"ones": ones, "eps": eps_t, "wpool": wpool,
              "tmp": tmp, "stats": stats, "bcast": bcast, "srow": srow,
              "pexp": pexp, "pmm": pmm, "pscore": pscore, "ppv": ppv}

        x16_r = x16.ap().rearrange("(dt dp) t -> dp dt t", dp=P)
        c16_r = ctx16.ap().rearrange("(dt dp) t -> dp dt t", dp=P)
        m01_r = m01.ap().rearrange("(kt kp) q -> kp kt q", kp=P)

        # ---- phase A: stats + applies + qkv ------------------------------
        # pool stacks are LIFO per side; open in reverse death order
        pxb = popen("pxb", "right")
        pxa = popen("pxa", "right")
        psa = popen("psa", "right")
        phc8 = popen("phc8", "left")
        pcatt1 = popen("pcatt1", "left")
        px = popen("px", "left")
        pattn1 = popen("pattn1", "left")
        pc = popen("pc", "left")
        ph = popen("ph", "left")

        xs = px.tile([P, 8, L], F16, tag="xs")
        cs16 = pc.tile([P, 8, MCTX], F16, tag="cs16")
        for c in range(2):
            nc.sync.dma_start(out=xs[:, :, c * 512:c * 512 + 512],
                              in_=x16_r[:, :, c * 512:c * 512 + 512])
            nc.sync.dma_start(out=cs16[:, :, c * 512:c * 512 + 512],
                              in_=c16_r[:, :, c * 512:c * 512 + 512])
        m01t = const.tile([P, 4, Q], F16)
        nc.sync.dma_start(out=m01t, in_=m01_r)

        ab_x = [_stats(nc, pp, xs, c) for c in range(2)]
        ab_c = [_stats(nc, pp, cs16, c) for c in range(2)]

        h16 = ph.tile([P, 8, Q], F16, tag="h16")   # LN(x) own chunk (q-proj)
        h8 = ph.tile([P, 8, L], F8, tag="h8")
        _apply(nc, pp, xs, 0, *ab_x[0], h16, h8)
        _apply(nc, pp, xs, 1, *ab_x[1], None, h8)
        hc8 = phc8.tile([P, 8, MCTX], F8, tag="hc8")

        qT = pattn1.tile([P, 8, Q], F16, tag="qT")
        kT = pattn1.tile([P, 8, L], F16, tag="kT")
        vt = pattn1.tile([P, 8, NH, HD + 1], F8, tag="vt")
        nc.vector.memset(vt[:, :, :, HD:HD + 1], 1.0)

        for ft, ps in _proj16(nc, pp, WqT, h16):
            nc.vector.tensor_copy(qT[:, ft, :], ps)

        wk_ap = Wk8.ap().rearrange("(dt dp) f -> dp dt f", dp=P)
        for c in range(2):
            wc = wpool.tile([P, 8, 512], F8, tag="w8", name=f"wk_{c}")
            nc.sync.dma_start(out=wc, in_=wk_ap[:, :, c * 512:c * 512 + 512])
            for fs in range(4):
                ft = c * 4 + fs
                for ch in range(2):
                    ps = pmm.tile([P, 512], F32, tag="mm", name=f"k_{ft}_{ch}")
                    for j in range(4):
                        nc.tensor.matmul(
                            ps, wc[:, 2 * j:2 * j + 2, fs * P:fs * P + P],
                            h8[:, 2 * j:2 * j + 2, ch * 512:ch * 512 + 512],
                            start=(j == 0), stop=(j == 3), perf_mode=DR)
                    nc.vector.tensor_scalar(
                        kT[:, ft, ch * 512:ch * 512 + 512], ps, 1.0 / WS,
                        None, ALU.mult)

        wv_ap = Wv8.ap().rearrange("(dt dp) f -> dp dt f", dp=P)
        for c in range(2):
            wc = wpool.tile([P, 8, 512], F8, tag="w8", name=f"wv_{c}")
            nc.sync.dma_start(out=wc, in_=wv_ap[:, :, c * 512:c * 512 + 512])
            for tt in range(8):
                ps = pmm.tile([P, 512], F32, tag="mm", name=f"v_{c}_{tt}")
                for j in range(4):
                    nc.tensor.matmul(
                        ps, h8[:, 2 * j:2 * j + 2, tt * P:tt * P + P],
                        wc[:, 2 * j:2 * j + 2, :],
                        start=(j == 0), stop=(j == 3), perf_mode=DR)
                nc.vector.tensor_scalar(
                    vt[:, tt, c * 8:c * 8 + 8, 0:HD],
                    ps.rearrange("p (h d) -> p h d", h=8), 1.0 / WS,
                    None, ALU.mult)
        pclose("ph")

        # ctx applies AFTER the q/k/v evictions in the DVE queue: they are
        # only needed by the k2/v2 fillers, and queueing them earlier would
        # stall the attention-critical DVE/PE chain behind them.
        for c in range(2):
            _apply(nc, pp, cs16, c, *ab_c[c], None, hc8)
        pclose("pc")

        # ---- k2/v2 as filler units (run inside ACT-bound attention) ------
        k2T = pcatt1.tile([P, 8, MCTX], F16, tag="k2T")
        v2t = pcatt1.tile([P, 8, NH, HD + 1], F8, tag="v2t")
        nc.vector.memset(v2t[:, :, :, HD:HD + 1], 1.0)
        wk2_ap = Wk28.ap().rearrange("(dt dp) f -> dp dt f", dp=P)
        wv2_ap = Wv28.ap().rearrange("(dt dp) f -> dp dt f", dp=P)
        wch = {}

        def mk_k2(c, fs, ch):
            def f():
                if fs == 0:
                    wc = wpool.tile([P, 8, 512], F8, tag="w8",
                                    name=f"wk2_{c}_{ch}")
                    nc.sync.dma_start(
                        out=wc, in_=wk2_ap[:, :, c * 512:c * 512 + 512])
                    wch["k", c] = wc
                wc = wch["k", c]
                ft = c * 4 + fs
                ps = pmm.tile([P, 512], F32, tag="mm", name=f"k2_{ft}_{ch}")
                for j in range(4):
                    nc.tensor.matmul(
                        ps, wc[:, 2 * j:2 * j + 2, fs * P:fs * P + P],
                        hc8[:, 2 * j:2 * j + 2, ch * 512:ch * 512 + 512],
                        start=(j == 0), stop=(j == 3), perf_mode=DR)
                nc.vector.tensor_scalar(
                    k2T[:, ft, ch * 512:ch * 512 + 512], ps, 1.0 / WS,
                    None, ALU.mult)
            return f

        def mk_v2(c, tt):
            def f():
                if tt % 4 == 0:
                    wc = wpool.tile([P, 8, 512], F8, tag="w8",
                                    name=f"wv2_{c}_{tt}")
                    nc.sync.dma_start(
                        out=wc, in_=wv2_ap[:, :, c * 512:c * 512 + 512])
                    wch["v", c] = wc
                wc = wch["v", c]
                ps = pmm.tile([P, 512], F32, tag="mm", name=f"v2_{c}_{tt}")
                for j in range(4):
                    nc.tensor.matmul(
                        ps, hc8[:, 2 * j:2 * j + 2, tt * P:tt * P + P],
                        wc[:, 2 * j:2 * j + 2, :],
                        start=(j == 0), stop=(j == 3), perf_mode=DR)
                nc.vector.tensor_scalar(
                    v2t[:, tt, c * 8:c * 8 + 8, 0:HD],
                    ps.rearrange("p (h d) -> p h d", h=8), 1.0 / WS,
                    None, ALU.mult)
            return f

        # hc8-chunk-0 consumers first: they unblock earliest during attention
        units = ([mk_k2(c, fs, 0) for c in range(2) for fs in range(4)]
                 + [mk_v2(c, tt) for c in range(2) for tt in range(4)]
                 + [mk_k2(c, fs, 1) for c in range(2) for fs in range(4)]
                 + [mk_v2(c, tt) for c in range(2) for tt in range(4, 8)])

        # ---- self-attention + out-proj + residual ------------------------
        saT = psa.tile([P, 8, Q], F16, tag="saT")
        _attention(nc, pp, kT, vt, qT, saT, m01t, tb_t, fillers=units[:16])

        xa16 = pxa.tile([P, 8, Q], F16, tag="xa16")
        for ft, ps in _proj16(nc, pp, WsoT, saT):
            nc.vector.tensor_add(xa16[:, ft, :], ps, xs[:, ft, 0:Q])
        pclose("psa")
        pclose("pattn1")
        pclose("px")

        # ---- cross-attention ---------------------------------------------
        for f in units[16:24]:
            f()
        A_xa, B_xa = _stats(nc, pp, xa16, 0)
        for f in units[24:]:
            f()

        pq2 = popen("pq2", "left")
        q2T = pq2.tile([P, 8, Q], F16, tag="q2T")
        phq = popen("phq", "left")
        hq16 = phq.tile([P, 8, Q], F16, tag="hq16")
        _apply(nc, pp, xa16, 0, A_xa, B_xa, hq16)
        for ft, ps in _proj16(nc, pp, Wq2T, hq16):
            nc.vector.tensor_copy(q2T[:, ft, :], ps)
        pclose("phq")

        pca = popen("pca", "right")
        caT = pca.tile([P, 8, Q], F16, tag="caT")
        _attention(nc, pp, k2T, v2t, q2T, caT, None, None)

        xb16 = pxb.tile([P, 8, Q], F16, tag="xb16")
        for ft, ps in _proj16(nc, pp, WcoT, caT):
            nc.vector.tensor_add(xb16[:, ft, :], ps, xa16[:, ft, :])
        pclose("pca")
        pclose("pq2")
        pclose("pcatt1")
        pclose("phc8")
        pclose("pxa")

        # ---- MLP ----------------------------------------------------------
        A_xb, B_xb = _stats(nc, pp, xb16, 0)
        pmlp = popen("pmlp", "left")
        h28 = pmlp.tile([P, 8, Q], F8, tag="h28")
        _apply(nc, pp, xb16, 0, A_xb, B_xb, None, h28)

        gt = pmlp.tile([P, 32, Q], F8, tag="gt")
        w1_ap = W18.ap().rearrange("(dt dp) f -> dp dt f", dp=P)
        for c in range(8):
            wc = wpool.tile([P, 8, 512], F8, tag="w8", name=f"w1_{c}")
            nc.sync.dma_start(out=wc, in_=w1_ap[:, :, c * 512:c * 512 + 512])
            for fs in range(4):
                ps = pmm.tile([P, 512], F32, tag="mm", name=f"f1_{c}_{fs}")
                for j in range(4):
                    nc.tensor.matmul(
                        ps, wc[:, 2 * j:2 * j + 2, fs * P:fs * P + P],
                        h28[:, 2 * j:2 * j + 2, :],
                        start=(j == 0), stop=(j == 3), perf_mode=DR)
                nc.scalar.activation(gt[:, c * 4 + fs, :], ps, AFT.Gelu,
                                     scale=1.0 / WS)

        ot = pmlp.tile([P, 8, Q], F32, tag="ot")
        w2_ap = W28.ap().rearrange("(dt dp) f -> dp dt f", dp=P)
        outT_r = outT.ap().rearrange("(dt dp) q -> dp dt q", dp=P)
        for ch in range(2):
            for half in range(2):
                pss = [pmm.tile([P, 512], F32, tag="mm",
                                name=f"f2_{ch}_{half}_{e}") for e in range(2)]
                for g in range(4):
                    wc = wpool.tile([P, 8, 512], F8, tag="w8",
                                    name=f"w2_{ch}_{half}_{g}")
                    nc.sync.dma_start(
                        out=wc, in_=w2_ap[:, g * 8:g * 8 + 8,
                                          ch * 512:ch * 512 + 512])
                    for e in range(2):
                        for j in range(4):
                            nc.tensor.matmul(
                                pss[e],
                                wc[:, 2 * j:2 * j + 2,
                                   (half * 2 + e) * P:(half * 2 + e + 1) * P],
                                gt[:, g * 8 + 2 * j:g * 8 + 2 * j + 2, :],
                                start=(g == 0 and j == 0),
                                stop=(g == 3 and j == 3), perf_mode=DR)
                for e in range(2):
                    ft = ch * 4 + half * 2 + e
                    nc.vector.scalar_tensor_tensor(
                        ot[:, ft, :], pss[e], 1.0 / WS2, xb16[:, ft, :],
                        ALU.mult, ALU.add)
                    nc.sync.dma_start(out=outT_r[:, ft, :], in_=ot[:, ft, :])
        pclose("pxb")
        pclose("pmlp")

    nc.compile()
    return nc


# ----------------------------------------------------------------------------
# host side
# ----------------------------------------------------------------------------

def _prep_inputs(x, context, sa_mask, W_qkv, W_self_out, W_q, W_kv, W_cross_out,
                 W_fc1, W_fc2, g_norm1, g_query_norm, g_context_norm, g_norm2):
    f32, f16 = np.float32, np.float16
    f8 = mybir.dt.np(F8)
    g1 = np.asarray(g_norm1, f32)[:, None]
    gq = np.asarray(g_query_norm, f32)[:, None]
    gc = np.asarray(g_context_norm, f32)[:, None]
    g2 = np.asarray(g_norm2, f32)[:, None]
    W_qkv = np.asarray(W_qkv, f32)
    W_kv = np.asarray(W_kv, f32)
    cw = lambda a: np.ascontiguousarray(a.astype(f16))
    cw8 = lambda a, s: np.ascontiguousarray((a * f32(s)).astype(f8))
    weights = {
        "WqT": cw(W_qkv[0:D].T * g1 * f32(SCALE)),
        "Wk8": cw8(W_qkv[D:2 * D].T * g1, WS),
        "Wv8": cw8(W_qkv[2 * D:3 * D].T * g1, WS),
        "WsoT": cw(np.asarray(W_self_out, f32).T),
        "Wq2T": cw(np.asarray(W_q, f32).T * gq * f32(SCALE)),
        "Wk28": cw8(W_kv[0:D].T * gc, WS),
        "Wv28": cw8(W_kv[D:2 * D].T * gc, WS),
        "WcoT": cw(np.asarray(W_cross_out, f32).T),
        "W18": cw8(np.asarray(W_fc1, f32).T * g2, WS),
        "W28": cw8(np.asarray(W_fc2, f32).T, WS2),
    }
    in_maps = []
    for c in range(8):
        b, s = c // 2, c % 2
        own = np.arange(s * Q, s * Q + Q)
        idx = np.concatenate([own, np.arange((1 - s) * Q, (1 - s) * Q + Q)])
        xb = np.asarray(x[b], f32)
        m01 = (np.asarray(sa_mask[b])[np.ix_(own, own)] != 0).astype(f16)
        m = dict(weights)
        m["x16"] = np.ascontiguousarray(xb[idx].T.astype(f16))
        m["m01"] = np.ascontiguousarray(m01.T)
        m["tbias"] = np.full((P, 1), NEG if s == 0 else 0.0, f32)
        m["ctx16"] = np.ascontiguousarray(
            np.asarray(context[b], f32).T.astype(f16))
        in_maps.append(m)
    return in_maps


def _check_mask(sa_mask):
    mask = np.asarray(sa_mask)
    lo, hi = np.arange(0, Q), np.arange(Q, L)
    for b in range(B):
        if not np.all(mask[b][np.ix_(lo, hi)] == 0):
            return False
        if not np.all(mask[b][np.ix_(hi, lo)] != 0):
            return False
    return True


def _gather(results, x_dtype):
    out = np.empty((B, L, D), np.float32)
    for c in range(8):
        b, s = c // 2, c % 2
        out[b, s * Q:(s + 1) * Q, :] = results[c]["outT"].T
    return out.astype(x_dtype, copy=False)


def _run(trace=False, **inputs):
    assert _check_mask(inputs["sa_mask"]), \
        "sa_mask does not have the expected causal block structure"
    if "nc" not in _CACHE:
        _CACHE["nc"] = build_program()
    nc = _CACHE["nc"]
    in_maps = _prep_inputs(**inputs)
    res = run_bass_kernel_spmd(nc, in_maps, list(range(8)), trace=trace)
    out = _gather(res.results, np.asarray(inputs["x"]).dtype)
    return out, res


def kernel(**inputs) -> np.ndarray:
    out, _ = _run(trace=False, **inputs)
    return out


def kernel_traced(**inputs):
    """Returns (output, exec_time_ns). Used by test.py."""
    import sys, types
    try:
        import antenv
        import trn_agent_boot.trn_boot as tb
        import concourse.bass_utils as bu
        if "antenv.axon_hooks" not in sys.modules:
            hook = tb._ntff_profile_via_ctypes('/opt/axon/libaxon_pjrt.so')
            mod = types.ModuleType("antenv.axon_hooks")
            mod.get_axon_ntff_profile_hook = lambda: hook
            mod.set_axon_ntff_profile_hook = lambda h: None
            sys.modules['antenv.axon_hooks'] = mod
            antenv.axon_hooks = mod
        bu.upload_artifacts = lambda tmpdir: "local://skipped"
    except Exception as e:
        print(f"ntff hook install failed: {e}")
    out, res = _run(trace=True, **inputs)
    return out, res.exec_time_ns


# revision 31
# speedup vs baseline: 1.2992x; 1.2992x over previous
"""Trainium2 Bass kernel for a transformer decoder block (self-attn + cross-attn + MLP).

Sharding: 8 cores = 4 batches x 2 sequence-halves; each core computes the full
block for its 512 query tokens (k/v over the full sequence; cross k/v over the
full context). Per-core token rotation puts own tokens first so one uniform
SPMD program serves both halves (causal mask = explicit triangle for own-half
keys + per-core scalar exp-bias for other-half keys).

Pipeline design (vs the 703us baseline):
- LayerNorm never blocks the PE: stats are interleaved ones-matmuls; the
  per-token affine (a,b rows via ACT Rsqrt + one fused scalar_tensor_tensor)
  is applied by DVE while the PE continues with the next matmul stream. The
  PE p-state ramp (0.65/1.2/2.4 GHz, 3us to full clock) makes continuous
  streams worth ~1.3x on their own.
- fp8 e4m3 DoubleRow matmuls (2 contraction rows/cycle) for the k/v/k2/v2
  projections, fc1/fc2, and P*V, with x32 (x64 fc2) weight pre-scaling to
  dodge fp8 subnormals; the descale folds into existing evictions (tensor
  _scalar) or the gelu activation scale. q/so/q2/co + scores stay fp16.
- P*V computed transposed (out [q,hd] per head, full 128x128 PE util) with a
  ones column in v giving the softmax denominator; per-head [128,4] batched
  reciprocal; PE transposes (identity matmul) restore feature-major.
- fp16 residual stream; softmax runs without max-subtraction (logits <= ~3,
  pexp <= e^3 << fp8 max 448; masked entries underflow exp to exactly 0).
"""

import numpy as np
from contextlib import ExitStack

import concourse.bass as bass
import concourse.tile as tile
from concourse import bacc, mybir
from concourse.bass_utils import run_bass_kernel_spmd

F32 = mybir.dt.float32
F16 = mybir.dt.float16
F8 = mybir.dt.float8e4
AFT = mybir.ActivationFunctionType
ALU = mybir.AluOpType
DR = mybir.MatmulPerfMode.DoubleRow

B, L, D = 4, 1024, 1024
MCTX = 1024
NH, HD = 16, 64
HID = 4 * D
EPS = 1e-6
SCALE = HD ** -0.5
Q = 512
P = 128
NEG = -30000.0
WS = 32.0    # fp8 weight pre-scale (projections, fc1)
WS2 = 64.0   # fp8 weight pre-scale (fc2)

_CACHE = {}


def _stats(nc, pp, src, ch):
    """LN stats over features for one 512-token chunk of src [128,8,width] f16.
    Returns (A, B) [128,512] f16 broadcast tiles: LN(x) = x*A + B."""
    pmm, tmp, st, bc = pp["pmm"], pp["tmp"], pp["stats"], pp["bcast"]
    ones, eps_t = pp["ones"], pp["eps"]
    cs = slice(ch * 512, ch * 512 + 512)
    ps_s = pmm.tile([P, 512], F32, tag="mm", name=f"st_s")
    ps_q = pmm.tile([P, 512], F32, tag="mm", name=f"st_q")
    for dt in range(8):
        nc.tensor.matmul(ps_s[0:1, :], ones, src[:, dt, cs],
                         start=(dt == 0), stop=(dt == 7))
        sq = tmp.tile([P, 512], F16, tag="sq", bufs=2)
        nc.vector.tensor_mul(sq, src[:, dt, cs], src[:, dt, cs])
        nc.tensor.matmul(ps_q[0:1, :], ones, sq,
                         start=(dt == 0), stop=(dt == 7))
    m2 = st.tile([1, 512], F32, tag="r32", bufs=3, name="m2")
    nc.scalar.activation(m2, ps_s[0:1, :], AFT.Square)
    v2 = st.tile([1, 512], F32, tag="r32", bufs=3, name="v2")
    nc.vector.scalar_tensor_tensor(v2, m2, -1.0 / D, ps_q[0:1, :],
                                   ALU.mult, ALU.add)
    sd = st.tile([1, 512], F32, tag="r32", bufs=3, name="sd")
    nc.scalar.activation(sd, v2, AFT.Sqrt, bias=eps_t, scale=1.0 / D)
    a = st.tile([1, 512], F32, tag="r32", bufs=3, name="a")
    nc.vector.reciprocal_approx_fast(a, sd)
    bm = st.tile([1, 512], F32, tag="r32", bufs=3, name="bm")
    nc.vector.scalar_tensor_tensor(bm, ps_s[0:1, :], -1.0 / D, a,
                                   ALU.mult, ALU.mult)
    a16 = st.tile([1, 512], F16, tag="r16", bufs=2, name="a16")
    nc.vector.tensor_copy(a16, a)
    b16 = st.tile([1, 512], F16, tag="r16", bufs=2, name="b16")
    nc.vector.tensor_copy(b16, bm)
    A = bc.tile([P, 512], F16, tag="A")
    nc.gpsimd.partition_broadcast(A, a16)
    Bt = bc.tile([P, 512], F16, tag="B")
    nc.gpsimd.partition_broadcast(Bt, b16)
    return A, Bt


def _apply(nc, pp, src, ch, A, Bt, dst16, dst8=None):
    """LN apply: dst[:,dt,cs] = src[:,dt,cs]*A + B. dst16 f16 (may be None),
    dst8 optional f8 twin (cast via tmp ring when dst16 is None)."""
    tmp = pp["tmp"]
    cs = slice(ch * 512, ch * 512 + 512)
    for dt in range(8):
        t1 = tmp.tile([P, 512], F16, tag="ap1")
        nc.vector.tensor_mul(t1, src[:, dt, cs], A)
        if dst16 is not None:
            nc.vector.tensor_add(dst16[:, dt, cs], t1, Bt)
            if dst8 is not None:
                nc.vector.tensor_copy(dst8[:, dt, cs], dst16[:, dt, cs])
        else:
            t2 = tmp.tile([P, 512], F16, tag="ap2", bufs=2)
            nc.vector.tensor_add(t2, t1, Bt)
            nc.vector.tensor_copy(dst8[:, dt, cs], t2)


def _proj16(nc, pp, w_dram, rhs, rhs_cols=None):
    """fp16 projection: yields (ft, psum [128,512]) for 8 f-tiles."""
    wpool, pmm = pp["wpool"], pp["pmm"]
    w_ap = w_dram.ap().rearrange("(dt dp) f -> dp dt f", dp=P)
    rc = rhs_cols if rhs_cols is not None else slice(0, 512)
    for c in range(2):
        wc = wpool.tile([P, 8, 512], F16, tag="w16", bufs=2)
        nc.sync.dma_start(out=wc, in_=w_ap[:, :, c * 512:c * 512 + 512])
        for fs in range(4):
            ft = c * 4 + fs
            ps = pmm.tile([P, 512], F32, tag="mm", name=f"p16_{ft}")
            for dt in range(8):
                nc.tensor.matmul(ps, wc[:, dt, fs * P:fs * P + P],
                                 rhs[:, dt, rc], start=(dt == 0), stop=(dt == 7))
            yield ft, ps


def _attention(nc, pp, kT, vt, qT, saT, m01t, tb_t, fillers=None):
    """Attention with feature-major PV: out [65,512] per head (row 64 = softmax
    denominator via the ones column of vt). kT f16 [128,8,1024], vt f8
    [128,8,16,65], qT f16 [128,8,512] -> saT f16 [128,8,512].
    m01t/tb_t non-None => causal self-attention (rotated layout): score
    matmuls + exp skip the fully-masked prefix of own-half k-tile pairs.
    fillers: callables popped one per head to emit independent PE work into
    the ACT/DVE-bound stretches."""
    pscore, ppv = pp["pscore"], pp["ppv"]
    pexp, tmp, srow, bc = pp["pexp"], pp["tmp"], pp["srow"], pp["bcast"]
    masked = m01t is not None
    for h in range(NH):
        ft, fo = h // 2, (h % 2) * HD
        pe_h = pexp.tile([P, 8, 512], F8, tag="pexp", name=f"pe_{h}")
        if masked:
            nc.vector.memset(pe_h[:, 2:4, 0:256], 0.0)
        for t in range(4):
            # live q-range union of k-tile pair (2t, 2t+1) under causal mask
            q0 = 256 * t if (masked and t < 2) else 0
            ps2 = pscore.tile([P, 2, 512], F32, tag="sc", name=f"sc_{h}_{t}")
            for i in range(2):
                kt = 2 * t + i
                nc.tensor.matmul(
                    ps2[:, i, q0:512],
                    kT[fo:fo + HD, ft, kt * P:kt * P + P],
                    qT[fo:fo + HD, ft, q0:512], start=True, stop=True)
            if masked and t < 2:
                et = tmp.tile([P, 2, 512], F16, tag="et", bufs=2)
                nc.scalar.activation(et[:, :, q0:], ps2[:, :, q0:], AFT.Exp)
                nc.vector.tensor_mul(pe_h[:, 2 * t:2 * t + 2, q0:],
                                     et[:, :, q0:],
                                     m01t[:, 2 * t:2 * t + 2, q0:])
            elif masked:
                nc.scalar.activation(pe_h[:, 2 * t:2 * t + 2, :], ps2,
                                     AFT.Exp, bias=tb_t)
            else:
                nc.scalar.activation(pe_h[:, 2 * t:2 * t + 2, :], ps2,
                                     AFT.Exp)
        pvp = ppv.tile([P, 512], F32, tag="pv", name=f"pv_{h}")
        for t in range(4):
            nc.tensor.matmul(pvp[0:HD + 1, :], vt[:, 2 * t:2 * t + 2, h, :],
                             pe_h[:, 2 * t:2 * t + 2, :],
                             start=(t == 0), stop=(t == 3), perf_mode=DR)
        dn = srow.tile([1, 512], F32, tag="dn", bufs=1, name=f"dn_{h}")
        nc.vector.tensor_copy(dn, pvp[HD:HD + 1, :])
        rb = srow.tile([1, 512], F32, tag="rb", bufs=1, name=f"rb_{h}")
        nc.vector.reciprocal_approx_fast(rb, dn)
        rb16 = srow.tile([1, 512], F16, tag="rb16", bufs=1, name=f"rb16_{h}")
        nc.vector.tensor_copy(rb16, rb)
        rbc = bc.tile([HD, 512], F16, tag="rbc", name=f"rbc_{h}")
        nc.gpsimd.partition_broadcast(rbc, rb16)
        nc.vector.tensor_mul(saT[fo:fo + HD, ft, :], pvp[0:HD, :], rbc)
        if fillers:
            fillers.pop(0)()


def build_program():
    nc = bacc.Bacc("TRN2", target_bir_lowering=False, debug=False,
                   enable_asserts=False)

    din = lambda n, shape, dt_=F16: nc.declare_dram_parameter(
        n, shape, dt_, isOutput=False)
    x16 = din("x16", [D, L])             # rotated, feature-major
    ctx16 = din("ctx16", [D, MCTX])
    m01 = din("m01", [Q, Q])             # own-half 0/1 causal mask [keys, q]
    tbias = din("tbias", [P, 1], F32)    # 0 (s=1) or -30000 (s=0) tail bias
    WqT, WsoT = din("WqT", [D, D]), din("WsoT", [D, D])
    Wq2T, WcoT = din("Wq2T", [D, D]), din("WcoT", [D, D])
    Wk8, Wv8 = din("Wk8", [D, D], F8), din("Wv8", [D, D], F8)
    Wk28, Wv28 = din("Wk28", [D, D], F8), din("Wv28", [D, D], F8)
    W18, W28 = din("W18", [D, HID], F8), din("W28", [HID, D], F8)
    outT = nc.declare_dram_parameter("outT", [D, Q], F32, isOutput=True)

    es = {}
    with tile.TileContext(nc) as tc, ExitStack() as top:
        def popen(name, side, bufs=1, **kw):
            s = ExitStack()
            es[name] = s
            return s.enter_context(
                tc.tile_pool(name=name, bufs=bufs, side=side, **kw))

        def pclose(name):
            es.pop(name).close()

        const = top.enter_context(tc.tile_pool(name="const", bufs=1))
        wpool = top.enter_context(tc.tile_pool(name="wpool", bufs=3))
        tmp = top.enter_context(tc.tile_pool(name="tmp", bufs=3))
        stats = top.enter_context(tc.tile_pool(name="stats", bufs=4))
        bcast = top.enter_context(tc.tile_pool(name="bcast", bufs=2))
        srow = top.enter_context(tc.tile_pool(name="srow", bufs=2))
        pexp = top.enter_context(tc.tile_pool(name="pexp", bufs=2))
        pmm = top.enter_context(
            tc.tile_pool(name="pmm", bufs=2, space="PSUM"))
        pscore = top.enter_context(
            tc.tile_pool(name="pscore", bufs=2, space="PSUM"))
        ppv = top.enter_context(
            tc.tile_pool(name="ppv", bufs=2, space="PSUM"))

        ones = const.tile([P, 1], F16)
        nc.vector.memset(ones, 1.0)
        eps_t = const.tile([1, 1], F32)
        nc.vector.memset(eps_t, EPS)
        tb_t = const.tile([P, 1], F32)
        nc.sync.dma_start(out=tb_t, in_=tbias[:, :])

        pp = {"ones": ones, "eps": eps_t, "wpool": wpool,
              "tmp": tmp, "stats": stats, "bcast": bcast, "srow": srow,
              "pexp": pexp, "pmm": pmm, "pscore": pscore, "ppv": ppv}

        x16_r = x16.ap().rearrange("(dt dp) t -> dp dt t", dp=P)
        c16_r = ctx16.ap().rearrange("(dt dp) t -> dp dt t", dp=P)
        m01_r = m01.ap().rearrange("(kt kp) q -> kp kt q", kp=P)

        # ---- phase A: stats + applies + qkv ------------------------------
        # pool stacks are LIFO per side; open in reverse death order
        pxb = popen("pxb", "right")
        pxa = popen("pxa", "right")
        psa = popen("psa", "right")
        phc8 = popen("phc8", "left")
        pcatt1 = popen("pcatt1", "left")
        px = popen("px", "left")
        pattn1 = popen("pattn1", "left")
        pc = popen("pc", "left")
        ph = popen("ph", "left")

        xs = px.tile([P, 8, L], F16, tag="xs")
        cs16 = pc.tile([P, 8, MCTX], F16, tag="cs16")
        for c in range(2):
            nc.sync.dma_start(out=xs[:, :, c * 512:c * 512 + 512],
                              in_=x16_r[:, :, c * 512:c * 512 + 512])
            nc.sync.dma_start(out=cs16[:, :, c * 512:c * 512 + 512],
                              in_=c16_r[:, :, c * 512:c * 512 + 512])
        m01t = const.tile([P, 4, Q], F16)
        nc.sync.dma_start(out=m01t, in_=m01_r)

        ab_x = [_stats(nc, pp, xs, c) for c in range(2)]
        ab_c = [_stats(nc, pp, cs16, c) for c in range(2)]

        h16 = ph.tile([P, 8, Q], F16, tag="h16")   # LN(x) own chunk (q-proj)
        h8 = ph.tile([P, 8, L], F8, tag="h8")
        _apply(nc, pp, xs, 0, *ab_x[0], h16, h8)
        _apply(nc, pp, xs, 1, *ab_x[1], None, h8)
        hc8 = phc8.tile([P, 8, MCTX], F8, tag="hc8")

        qT = pattn1.tile([P, 8, Q], F16, tag="qT")
        kT = pattn1.tile([P, 8, L], F16, tag="kT")
        vt = pattn1.tile([P, 8, NH, HD + 1], F8, tag="vt")
        nc.vector.memset(vt[:, :, :, HD:HD + 1], 1.0)

        for ft, ps in _proj16(nc, pp, WqT, h16):
            nc.vector.tensor_copy(qT[:, ft, :], ps)

        wk_ap = Wk8.ap().rearrange("(dt dp) f -> dp dt f", dp=P)
        for c in range(2):
            wc = wpool.tile([P, 8, 512], F8, tag="w8", name=f"wk_{c}")
            nc.sync.dma_start(out=wc, in_=wk_ap[:, :, c * 512:c * 512 + 512])
            for fs in range(4):
                ft = c * 4 + fs
                for ch in range(2):
                    ps = pmm.tile([P, 512], F32, tag="mm", name=f"k_{ft}_{ch}")
                    for j in range(4):
                        nc.tensor.matmul(
                            ps, wc[:, 2 * j:2 * j + 2, fs * P:fs * P + P],
                            h8[:, 2 * j:2 * j + 2, ch * 512:ch * 512 + 512],
                            start=(j == 0), stop=(j == 3), perf_mode=DR)
                    nc.vector.tensor_scalar(
                        kT[:, ft, ch * 512:ch * 512 + 512], ps, 1.0 / WS,
                        None, ALU.mult)

        wv_ap = Wv8.ap().rearrange("(dt dp) f -> dp dt f", dp=P)
        for c in range(2):
            wc = wpool.tile([P, 8, 512], F8, tag="w8", name=f"wv_{c}")
            nc.sync.dma_start(out=wc, in_=wv_ap[:, :, c * 512:c * 512 + 512])
            for tt in range(8):
                ps = pmm.tile([P, 512], F32, tag="mm", name=f"v_{c}_{tt}")
                for j in range(4):
                    nc.tensor.matmul(
                        ps, h8[:, 2 * j:2 * j + 2, tt * P:tt * P + P],
                        wc[:, 2 * j:2 * j + 2, :],
                        start=(j == 0), stop=(j == 3), perf_mode=DR)
                nc.vector.tensor_scalar(
                    vt[:, tt, c * 8:c * 8 + 8, 0:HD],
                    ps.rearrange("p (h d) -> p h d", h=8), 1.0 / WS,
                    None, ALU.mult)
        pclose("ph")

        # ctx applies AFTER the q/k/v evictions in the DVE queue: they are
        # only needed by the k2/v2 fillers, and queueing them earlier would
        # stall the attention-critical DVE/PE chain behind them.
        for c in range(2):
            _apply(nc, pp, cs16, c, *ab_c[c], None, hc8)
        pclose("pc")

        # ---- k2/v2 as filler units (run inside ACT-bound attention) ------
        k2T = pcatt1.tile([P, 8, MCTX], F16, tag="k2T")
        v2t = pcatt1.tile([P, 8, NH, HD + 1], F8, tag="v2t")
        nc.vector.memset(v2t[:, :, :, HD:HD + 1], 1.0)
        wk2_ap = Wk28.ap().rearrange("(dt dp) f -> dp dt f", dp=P)
        wv2_ap = Wv28.ap().rearrange("(dt dp) f -> dp dt f", dp=P)
        wch = {}

        def mk_k2(c, fs, ch):
            def f():
                if fs == 0:
                    wc = wpool.tile([P, 8, 512], F8, tag="w8",
                                    name=f"wk2_{c}_{ch}")
                    nc.sync.dma_start(
                        out=wc, in_=wk2_ap[:, :, c * 512:c * 512 + 512])
                    wch["k", c] = wc
                wc = wch["k", c]
                ft = c * 4 + fs
                ps = pmm.tile([P, 512], F32, tag="mm", name=f"k2_{ft}_{ch}")
                for j in range(4):
                    nc.tensor.matmul(
                        ps, wc[:, 2 * j:2 * j + 2, fs * P:fs * P + P],
                        hc8[:, 2 * j:2 * j + 2, ch * 512:ch * 512 + 512],
                        start=(j == 0), stop=(j == 3), perf_mode=DR)
                nc.vector.tensor_scalar(
                    k2T[:, ft, ch * 512:ch * 512 + 512], ps, 1.0 / WS,
                    None, ALU.mult)
            return f

        def mk_v2(c, tt):
            def f():
                if tt % 4 == 0:
                    wc = wpool.tile([P, 8, 512], F8, tag="w8",
                                    name=f"wv2_{c}_{tt}")
                    nc.sync.dma_start(
                        out=wc, in_=wv2_ap[:, :, c * 512:c * 512 + 512])
                    wch["v", c] = wc
                wc = wch["v", c]
                ps = pmm.tile([P, 512], F32, tag="mm", name=f"v2_{c}_{tt}")
                for j in range(4):
                    nc.tensor.matmul(
                        ps, hc8[:, 2 * j:2 * j + 2, tt * P:tt * P + P],
                        wc[:, 2 * j:2 * j + 2, :],
                        start=(j == 0), stop=(j == 3), perf_mode=DR)
                nc.vector.tensor_scalar(
                    v2t[:, tt, c * 8:c * 8 + 8, 0:HD],
                    ps.rearrange("p (h d) -> p h d", h=8), 1.0 / WS,
                    None, ALU.mult)
            return f

        # hc8-chunk-0 consumers first: they unblock earliest during attention
        units = ([mk_k2(c, fs, 0) for c in range(2) for fs in range(4)]
                 + [mk_v2(c, tt) for c in range(2) for tt in range(4)]
                 + [mk_k2(c, fs, 1) for c in range(2) for fs in range(4)]
                 + [mk_v2(c, tt) for c in range(2) for tt in range(4, 8)])

        # ---- self-attention + out-proj + residual ------------------------
        saT = psa.tile([P, 8, Q], F16, tag="saT")
        _attention(nc, pp, kT, vt, qT, saT, m01t, tb_t, fillers=units[:16])

        xa16 = pxa.tile([P, 8, Q], F16, tag="xa16")
        for ft, ps in _proj16(nc, pp, WsoT, saT):
            nc.vector.tensor_add(xa16[:, ft, :], ps, xs[:, ft, 0:Q])
        pclose("psa")
        pclose("pattn1")
        pclose("px")

        # ---- cross-attention ---------------------------------------------
        for f in units[16:24]:
            f()
        A_xa, B_xa = _stats(nc, pp, xa16, 0)
        for f in units[24:]:
            f()

        pq2 = popen("pq2", "left")
        q2T = pq2.tile([P, 8, Q], F16, tag="q2T")
        phq = popen("phq", "left")
        hq16 = phq.tile([P, 8, Q], F16, tag="hq16")
        _apply(nc, pp, xa16, 0, A_xa, B_xa, hq16)
        for ft, ps in _proj16(nc, pp, Wq2T, hq16):
            nc.vector.tensor_copy(q2T[:, ft, :], ps)
        pclose("phq")

        pca = popen("pca", "right")
        caT = pca.tile([P, 8, Q], F16, tag="caT")
        _attention(nc, pp, k2T, v2t, q2T, caT, None, None)

        xb16 = pxb.tile([P, 8, Q], F16, tag="xb16")
        for ft, ps in _proj16(nc, pp, WcoT, caT):
            nc.vector.tensor_add(xb16[:, ft, :], ps, xa16[:, ft, :])
        pclose("pca")
        pclose("pq2")
        pclose("pcatt1")
        pclose("phc8")
        pclose("pxa")

        # ---- MLP ----------------------------------------------------------
        A_xb, B_xb = _stats(nc, pp, xb16, 0)
        pmlp = popen("pmlp", "left")
        h28 = pmlp.tile([P, 8, Q], F8, tag="h28")
        _apply(nc, pp, xb16, 0, A_xb, B_xb, None, h28)

        gt = pmlp.tile([P, 32, Q], F8, tag="gt")
        w1_ap = W18.ap().rearrange("(dt dp) f -> dp dt f", dp=P)
        for c in range(8):
            wc = wpool.tile([P, 8, 512], F8, tag="w8", name=f"w1_{c}")
            nc.sync.dma_start(out=wc, in_=w1_ap[:, :, c * 512:c * 512 + 512])
            for fs in range(4):
                ps = pmm.tile([P, 512], F32, tag="mm", name=f"f1_{c}_{fs}")
                for j in range(4):
                    nc.tensor.matmul(
                        ps, wc[:, 2 * j:2 * j + 2, fs * P:fs * P + P],
                        h28[:, 2 * j:2 * j + 2, :],
                        start=(j == 0), stop=(j == 3), perf_mode=DR)
                nc.scalar.activation(gt[:, c * 4 + fs, :], ps, AFT.Gelu,
                                     scale=1.0 / WS)

        ot = pmlp.tile([P, 8, Q], F32, tag="ot")
        w2_ap = W28.ap().rearrange("(dt dp) f -> dp dt f", dp=P)
        outT_r = outT.ap().rearrange("(dt dp) q -> dp dt q", dp=P)
        for ch in range(2):
            for half in range(2):
                pss = [pmm.tile([P, 512], F32, tag="mm",
                                name=f"f2_{ch}_{half}_{e}") for e in range(2)]
                for g in range(4):
                    wc = wpool.tile([P, 8, 512], F8, tag="w8",
                                    name=f"w2_{ch}_{half}_{g}")
                    nc.sync.dma_start(
                        out=wc, in_=w2_ap[:, g * 8:g * 8 + 8,
                                          ch * 512:ch * 512 + 512])
                    for e in range(2):
                        for j in range(4):
                            nc.tensor.matmul(
                                pss[e],
                                wc[:, 2 * j:2 * j + 2,
                                   (half * 2 + e) * P:(half * 2 + e + 1) * P],
                                gt[:, g * 8 + 2 * j:g * 8 + 2 * j + 2, :],
                                start=(g == 0 and j == 0),
                                stop=(g == 3 and j == 3), perf_mode=DR)
                for e in range(2):
                    ft = ch * 4 + half * 2 + e
                    nc.vector.scalar_tensor_tensor(
                        ot[:, ft, :], pss[e], 1.0 / WS2, xb16[:, ft, :],
                        ALU.mult, ALU.add)
                    nc.sync.dma_start(out=outT_r[:, ft, :], in_=ot[:, ft, :])
        pclose("pxb")
        pclose("pmlp")

    nc.compile()
    return nc


# ----------------------------------------------------------------------------
# host side
# ----------------------------------------------------------------------------

def _prep_inputs(x, context, sa_mask, W_qkv, W_self_out, W_q, W_kv, W_cross_out,
                 W_fc1, W_fc2, g_norm1, g_query_norm, g_context_norm, g_norm2):
    f32, f16 = np.float32, np.float16
    f8 = mybir.dt.np(F8)
    g1 = np.asarray(g_norm1, f32)[:, None]
    gq = np.asarray(g_query_norm, f32)[:, None]
    gc = np.asarray(g_context_norm, f32)[:, None]
    g2 = np.asarray(g_norm2, f32)[:, None]
    W_qkv = np.asarray(W_qkv, f32)
    W_kv = np.asarray(W_kv, f32)
    cw = lambda a: np.ascontiguousarray(a.astype(f16))
    cw8 = lambda a, s: np.ascontiguousarray((a * f32(s)).astype(f8))
    weights = {
        "WqT": cw(W_qkv[0:D].T * g1 * f32(SCALE)),
        "Wk8": cw8(W_qkv[D:2 * D].T * g1, WS),
        "Wv8": cw8(W_qkv[2 * D:3 * D].T * g1, WS),
        "WsoT": cw(np.asarray(W_self_out, f32).T),
        "Wq2T": cw(np.asarray(W_q, f32).T * gq * f32(SCALE)),
        "Wk28": cw8(W_kv[0:D].T * gc, WS),
        "Wv28": cw8(W_kv[D:2 * D].T * gc, WS),
        "WcoT": cw(np.asarray(W_cross_out, f32).T),
        "W18": cw8(np.asarray(W_fc1, f32).T * g2, WS),
        "W28": cw8(np.asarray(W_fc2, f32).T, WS2),
    }
    in_maps = []
    for c in range(8):
        b, s = c // 2, c % 2
        own = np.arange(s * Q, s * Q + Q)
        idx = np.concatenate([own, np.arange((1 - s) * Q, (1 - s) * Q + Q)])
        xb = np.asarray(x[b], f32)
        m01 = (np.asarray(sa_mask[b])[np.ix_(own, own)] != 0).astype(f16)
        m = dict(weights)
        m["x16"] = np.ascontiguousarray(xb[idx].T.astype(f16))
        m["m01"] = np.ascontiguousarray(m01.T)
        m["tbias"] = np.full((P, 1), NEG if s == 0 else 0.0, f32)
        m["ctx16"] = np.ascontiguousarray(
            np.asarray(context[b], f32).T.astype(f16))
        in_maps.append(m)
    return in_maps


def _check_mask(sa_mask):
    mask = np.asarray(sa_mask)
    lo, hi = np.arange(0, Q), np.arange(Q, L)
    for b in range(B):
        if not np.all(mask[b][np.ix_(lo, hi)] == 0):
            return False
        if not np.all(mask[b][np.ix_(hi, lo)] != 0):
            return False
    return True


def _gather(results, x_dtype):
    out = np.empty((B, L, D), np.float32)
    for c in range(8):
        b, s = c // 2, c % 2
        out[b, s * Q:(s + 1) * Q, :] = results[c]["outT"].T
    return out.astype(x_dtype, copy=False)


def _run(trace=False, **inputs):
    assert _check_mask(inputs["sa_mask"]), \
        "sa_mask does not have the expected causal block structure"
    if "nc" not in _CACHE:
        _CACHE["nc"] = build_program()
    nc = _CACHE["nc"]
    in_maps = _prep_inputs(**inputs)
    res = run_bass_kernel_spmd(nc, in_maps, list(range(8)), trace=trace)
    out = _gather(res.results, np.asarray(inputs["x"]).dtype)
    return out, res


def kernel(**inputs) -> np.ndarray:
    out, _ = _run(trace=False, **inputs)
    return out


def kernel_traced(**inputs):
    """Returns (output, exec_time_ns). Used by test.py."""
    import sys, types
    try:
        import antenv
        import trn_agent_boot.trn_boot as tb
        import concourse.bass_utils as bu
        if "antenv.axon_hooks" not in sys.modules:
            hook = tb._ntff_profile_via_ctypes('/opt/axon/libaxon_pjrt.so')
            mod = types.ModuleType("antenv.axon_hooks")
            mod.get_axon_ntff_profile_hook = lambda: hook
            mod.set_axon_ntff_profile_hook = lambda h: None
            sys.modules['antenv.axon_hooks'] = mod
            antenv.axon_hooks = mod
        bu.upload_artifacts = lambda tmpdir: "local://skipped"
    except Exception as e:
        print(f"ntff hook install failed: {e}")
    out, res = _run(trace=True, **inputs)
    return out, res.exec_time_ns


# revision 34
# speedup vs baseline: 1.3990x; 1.0769x over previous
"""Trainium2 Bass kernel for a transformer decoder block (self-attn + cross-attn + MLP).

Sharding: 8 cores = 4 batches x 2 sequence-halves; each core computes the full
block for its 512 query tokens (k/v over the full sequence; cross k/v over the
full context). Per-core token rotation puts own tokens first so one uniform
SPMD program serves both halves (causal mask = explicit triangle for own-half
keys + per-core scalar exp-bias for other-half keys).

Pipeline design (vs the 703us baseline):
- LayerNorm never blocks the PE: stats are interleaved ones-matmuls; the
  per-token affine (a,b rows via ACT Rsqrt + one fused scalar_tensor_tensor)
  is applied by DVE while the PE continues with the next matmul stream. The
  PE p-state ramp (0.65/1.2/2.4 GHz, 3us to full clock) makes continuous
  streams worth ~1.3x on their own.
- fp8 e4m3 DoubleRow matmuls (2 contraction rows/cycle) for the k/v/k2/v2
  projections, fc1/fc2, and P*V, with x32 (x64 fc2) weight pre-scaling to
  dodge fp8 subnormals; the descale folds into existing evictions (tensor
  _scalar) or the gelu activation scale. q/so/q2/co + scores stay fp16.
- P*V computed transposed (out [q,hd] per head, full 128x128 PE util) with a
  ones column in v giving the softmax denominator; per-head [128,4] batched
  reciprocal; PE transposes (identity matmul) restore feature-major.
- fp16 residual stream; softmax runs without max-subtraction (logits <= ~3,
  pexp <= e^3 << fp8 max 448; masked entries underflow exp to exactly 0).
"""

import numpy as np
from contextlib import ExitStack

import concourse.bass as bass
import concourse.tile as tile
from concourse import bacc, mybir
from concourse.bass_utils import run_bass_kernel_spmd

F32 = mybir.dt.float32
F16 = mybir.dt.float16
F8 = mybir.dt.float8e4
AFT = mybir.ActivationFunctionType
ALU = mybir.AluOpType
DR = mybir.MatmulPerfMode.DoubleRow

B, L, D = 4, 1024, 1024
MCTX = 1024
NH, HD = 16, 64
HID = 4 * D
EPS = 1e-6
SCALE = HD ** -0.5
Q = 512
P = 128
NEG = -30000.0
WS = 32.0    # fp8 weight pre-scale (projections, fc1)
WS2 = 64.0   # fp8 weight pre-scale (fc2)
WSQ = 256.0  # fp8 weight pre-scale (q/q2: 1/sqrt(64) folded in shrinks them)

_CACHE = {}


def _stats(nc, pp, src, ch):
    """LN stats over features for one 512-token chunk of src [128,8,width] f16.
    Returns (A, B) [128,512] f16 broadcast tiles: LN(x) = x*A + B."""
    pmm, tmp, st, bc = pp["pmm"], pp["tmp"], pp["stats"], pp["bcast"]
    ones, eps_t = pp["ones"], pp["eps"]
    cs = slice(ch * 512, ch * 512 + 512)
    ps_s = pmm.tile([P, 512], F32, tag="mm", name=f"st_s")
    ps_q = pmm.tile([P, 512], F32, tag="mm", name=f"st_q")
    for dt in range(8):
        nc.tensor.matmul(ps_s[0:1, :], ones, src[:, dt, cs],
                         start=(dt == 0), stop=(dt == 7))
        sq = tmp.tile([P, 512], F16, tag="sq", bufs=2)
        nc.vector.tensor_mul(sq, src[:, dt, cs], src[:, dt, cs])
        nc.tensor.matmul(ps_q[0:1, :], ones, sq,
                         start=(dt == 0), stop=(dt == 7))
    m2 = st.tile([1, 512], F32, tag="r32", bufs=3, name="m2")
    nc.scalar.activation(m2, ps_s[0:1, :], AFT.Square)
    v2 = st.tile([1, 512], F32, tag="r32", bufs=3, name="v2")
    nc.vector.scalar_tensor_tensor(v2, m2, -1.0 / D, ps_q[0:1, :],
                                   ALU.mult, ALU.add)
    sd = st.tile([1, 512], F32, tag="r32", bufs=3, name="sd")
    nc.scalar.activation(sd, v2, AFT.Sqrt, bias=eps_t, scale=1.0 / D)
    a = st.tile([1, 512], F32, tag="r32", bufs=3, name="a")
    nc.vector.reciprocal_approx_fast(a, sd)
    bm = st.tile([1, 512], F32, tag="r32", bufs=3, name="bm")
    nc.vector.scalar_tensor_tensor(bm, ps_s[0:1, :], -1.0 / D, a,
                                   ALU.mult, ALU.mult)
    a16 = st.tile([1, 512], F16, tag="r16", bufs=2, name="a16")
    nc.vector.tensor_copy(a16, a)
    b16 = st.tile([1, 512], F16, tag="r16", bufs=2, name="b16")
    nc.vector.tensor_copy(b16, bm)
    A = bc.tile([P, 512], F16, tag="A")
    nc.gpsimd.partition_broadcast(A, a16)
    Bt = bc.tile([P, 512], F16, tag="B")
    nc.gpsimd.partition_broadcast(Bt, b16)
    return A, Bt


def _apply(nc, pp, src, ch, A, Bt, dst16, dst8=None):
    """LN apply: dst[:,dt,cs] = src[:,dt,cs]*A + B. dst16 f16 (may be None),
    dst8 optional f8 twin (cast via tmp ring when dst16 is None)."""
    tmp = pp["tmp"]
    cs = slice(ch * 512, ch * 512 + 512)
    for dt in range(8):
        t1 = tmp.tile([P, 512], F16, tag="ap1")
        nc.vector.tensor_mul(t1, src[:, dt, cs], A)
        if dst16 is not None:
            nc.vector.tensor_add(dst16[:, dt, cs], t1, Bt)
            if dst8 is not None:
                nc.vector.tensor_copy(dst8[:, dt, cs], dst16[:, dt, cs])
        else:
            t2 = tmp.tile([P, 512], F16, tag="ap2", bufs=2)
            nc.vector.tensor_add(t2, t1, Bt)
            nc.vector.tensor_copy(dst8[:, dt, cs], t2)


def _proj8(nc, pp, w_dram, rhs8, name):
    """fp8 DoubleRow projection: yields (ft, psum [128,512]) for 8 f-tiles."""
    wpool, pmm = pp["wpool"], pp["pmm"]
    w_ap = w_dram.ap().rearrange("(dt dp) f -> dp dt f", dp=P)
    for c in range(2):
        wc = wpool.tile([P, 8, 512], F8, tag="w8", name=f"w_{name}_{c}")
        nc.sync.dma_start(out=wc, in_=w_ap[:, :, c * 512:c * 512 + 512])
        for fs in range(4):
            ft = c * 4 + fs
            ps = pmm.tile([P, 512], F32, tag="mm", name=f"p8_{name}_{ft}")
            for j in range(4):
                nc.tensor.matmul(ps, wc[:, 2 * j:2 * j + 2, fs * P:fs * P + P],
                                 rhs8[:, 2 * j:2 * j + 2, :],
                                 start=(j == 0), stop=(j == 3), perf_mode=DR)
            yield ft, ps


def _attention(nc, pp, kT, vt, qT, saT, m01t, tb_t,
               fillers=None, dunits=None):
    """Attention with feature-major PV: out [65,512] per head (row 64 = softmax
    denominator via the ones column of vt). kT f16 [128,8,1024], vt f8
    [128,8,16,65], qT f16 [128,8,512] -> saT f16 [128,8,512].
    m01t/tb_t non-None => causal self-attention (rotated layout): score
    matmuls + exp skip the fully-masked prefix of own-half k-tile pairs.
    fillers: callables popped one per head to emit independent PE work into
    the ACT/DVE-bound stretches."""
    pscore, ppv = pp["pscore"], pp["ppv"]
    pexp, tmp, srow, bc = pp["pexp"], pp["tmp"], pp["srow"], pp["bcast"]
    masked = m01t is not None
    for h in range(NH):
        ft, fo = h // 2, (h % 2) * HD
        pe_h = pexp.tile([P, 8, 512], F8, tag="pexp", name=f"pe_{h}")
        if masked:
            nc.vector.memset(pe_h[:, 2:4, 0:256], 0.0)
        for t in range(4):
            # live q-range union of k-tile pair (2t, 2t+1) under causal mask
            q0 = 256 * t if (masked and t < 2) else 0
            ps2 = pscore.tile([P, 2, 512], F32, tag="sc", name=f"sc_{h}_{t}")
            for i in range(2):
                kt = 2 * t + i
                nc.tensor.matmul(
                    ps2[:, i, q0:512],
                    kT[fo:fo + HD, ft, kt * P:kt * P + P],
                    qT[fo:fo + HD, ft, q0:512], start=True, stop=True)
            if masked and t < 2:
                et = tmp.tile([P, 2, 512], F16, tag="et", bufs=2)
                nc.scalar.activation(et[:, :, q0:], ps2[:, :, q0:], AFT.Exp)
                nc.vector.tensor_mul(pe_h[:, 2 * t:2 * t + 2, q0:],
                                     et[:, :, q0:],
                                     m01t[:, 2 * t:2 * t + 2, q0:])
            elif masked:
                nc.scalar.activation(pe_h[:, 2 * t:2 * t + 2, :], ps2,
                                     AFT.Exp, bias=tb_t)
            else:
                nc.scalar.activation(pe_h[:, 2 * t:2 * t + 2, :], ps2,
                                     AFT.Exp)
        pvp = ppv.tile([P, 512], F32, tag="pv", name=f"pv_{h}")
        for t in range(4):
            nc.tensor.matmul(pvp[0:HD + 1, :], vt[:, 2 * t:2 * t + 2, h, :],
                             pe_h[:, 2 * t:2 * t + 2, :],
                             start=(t == 0), stop=(t == 3), perf_mode=DR)
        dn = srow.tile([1, 512], F32, tag="dn", bufs=1, name=f"dn_{h}")
        nc.vector.tensor_copy(dn, pvp[HD:HD + 1, :])
        rb = srow.tile([1, 512], F32, tag="rb", bufs=1, name=f"rb_{h}")
        nc.vector.reciprocal_approx_fast(rb, dn)
        rb16 = srow.tile([1, 512], F16, tag="rb16", bufs=1, name=f"rb16_{h}")
        nc.vector.tensor_copy(rb16, rb)
        rbc = bc.tile([HD, 512], F16, tag="rbc", name=f"rbc_{h}")
        nc.gpsimd.partition_broadcast(rbc, rb16)
        nc.vector.tensor_mul(saT[fo:fo + HD, ft, :], pvp[0:HD, :], rbc)
        if fillers:
            fillers.pop(0)()
        for _ in range(2):
            if dunits:
                dunits.pop(0)()


def build_program():
    nc = bacc.Bacc("TRN2", target_bir_lowering=False, debug=False,
                   enable_asserts=False)

    din = lambda n, shape, dt_=F16: nc.declare_dram_parameter(
        n, shape, dt_, isOutput=False)
    x16 = din("x16", [D, L])             # rotated, feature-major
    ctx16 = din("ctx16", [D, MCTX])
    m01 = din("m01", [Q, Q])             # own-half 0/1 causal mask [keys, q]
    tbias = din("tbias", [P, 1], F32)    # 0 (s=1) or -30000 (s=0) tail bias
    Wq8, Wso8 = din("Wq8", [D, D], F8), din("Wso8", [D, D], F8)
    Wq28, Wco8 = din("Wq28", [D, D], F8), din("Wco8", [D, D], F8)
    Wk8, Wv8 = din("Wk8", [D, D], F8), din("Wv8", [D, D], F8)
    Wk28, Wv28 = din("Wk28", [D, D], F8), din("Wv28", [D, D], F8)
    W18, W28 = din("W18", [D, HID], F8), din("W28", [HID, D], F8)
    outT = nc.declare_dram_parameter("outT", [D, Q], F32, isOutput=True)

    es = {}
    with tile.TileContext(nc) as tc, ExitStack() as top:
        def popen(name, side, bufs=1, **kw):
            s = ExitStack()
            es[name] = s
            return s.enter_context(
                tc.tile_pool(name=name, bufs=bufs, side=side, **kw))

        def pclose(name):
            es.pop(name).close()

        const = top.enter_context(tc.tile_pool(name="const", bufs=1))
        wpool = top.enter_context(tc.tile_pool(name="wpool", bufs=3))
        tmp = top.enter_context(tc.tile_pool(name="tmp", bufs=3))
        stats = top.enter_context(tc.tile_pool(name="stats", bufs=4))
        bcast = top.enter_context(tc.tile_pool(name="bcast", bufs=2))
        srow = top.enter_context(tc.tile_pool(name="srow", bufs=2))
        pexp = top.enter_context(tc.tile_pool(name="pexp", bufs=2))
        pmm = top.enter_context(
            tc.tile_pool(name="pmm", bufs=2, space="PSUM"))
        pscore = top.enter_context(
            tc.tile_pool(name="pscore", bufs=2, space="PSUM"))
        ppv = top.enter_context(
            tc.tile_pool(name="ppv", bufs=2, space="PSUM"))

        ones = const.tile([P, 1], F16)
        nc.vector.memset(ones, 1.0)
        eps_t = const.tile([1, 1], F32)
        nc.vector.memset(eps_t, EPS)
        tb_t = const.tile([P, 1], F32)
        nc.sync.dma_start(out=tb_t, in_=tbias[:, :])

        pp = {"ones": ones, "eps": eps_t, "wpool": wpool,
              "tmp": tmp, "stats": stats, "bcast": bcast, "srow": srow,
              "pexp": pexp, "pmm": pmm, "pscore": pscore, "ppv": ppv}

        x16_r = x16.ap().rearrange("(dt dp) t -> dp dt t", dp=P)
        c16_r = ctx16.ap().rearrange("(dt dp) t -> dp dt t", dp=P)
        m01_r = m01.ap().rearrange("(kt kp) q -> kp kt q", kp=P)

        # ---- phase A: stats + applies + qkv ------------------------------
        # pool stacks are LIFO per side; open in reverse death order
        pxb = popen("pxb", "right")
        pxa = popen("pxa", "right")
        psa = popen("psa", "right")
        phc8 = popen("phc8", "left")
        pcatt1 = popen("pcatt1", "left")
        px = popen("px", "left")
        pattn1 = popen("pattn1", "left")
        pc = popen("pc", "left")
        ph = popen("ph", "left")

        xs = px.tile([P, 8, L], F16, tag="xs")
        cs16 = pc.tile([P, 8, MCTX], F16, tag="cs16")
        for c in range(2):
            nc.sync.dma_start(out=xs[:, :, c * 512:c * 512 + 512],
                              in_=x16_r[:, :, c * 512:c * 512 + 512])
            nc.sync.dma_start(out=cs16[:, :, c * 512:c * 512 + 512],
                              in_=c16_r[:, :, c * 512:c * 512 + 512])
        m01t = const.tile([P, 4, Q], F16)
        nc.sync.dma_start(out=m01t, in_=m01_r)

        ab_x = [_stats(nc, pp, xs, c) for c in range(2)]
        ab_c = [_stats(nc, pp, cs16, c) for c in range(2)]

        h8 = ph.tile([P, 8, L], F8, tag="h8")
        _apply(nc, pp, xs, 0, *ab_x[0], None, h8)
        _apply(nc, pp, xs, 1, *ab_x[1], None, h8)
        hc8 = phc8.tile([P, 8, MCTX], F8, tag="hc8")

        qT = pattn1.tile([P, 8, Q], F16, tag="qT")
        kT = pattn1.tile([P, 8, L], F16, tag="kT")
        vt = pattn1.tile([P, 8, NH, HD + 1], F8, tag="vt")
        nc.vector.memset(vt[:, :, :, HD:HD + 1], 1.0)

        for ft, ps in _proj8(nc, pp, Wq8, h8[:, :, 0:Q], "q"):
            nc.vector.tensor_scalar(qT[:, ft, :], ps, 1.0 / WSQ, None, ALU.mult)

        wk_ap = Wk8.ap().rearrange("(dt dp) f -> dp dt f", dp=P)
        for c in range(2):
            wc = wpool.tile([P, 8, 512], F8, tag="w8", name=f"wk_{c}")
            nc.sync.dma_start(out=wc, in_=wk_ap[:, :, c * 512:c * 512 + 512])
            for fs in range(4):
                ft = c * 4 + fs
                for ch in range(2):
                    ps = pmm.tile([P, 512], F32, tag="mm", name=f"k_{ft}_{ch}")
                    for j in range(4):
                        nc.tensor.matmul(
                            ps, wc[:, 2 * j:2 * j + 2, fs * P:fs * P + P],
                            h8[:, 2 * j:2 * j + 2, ch * 512:ch * 512 + 512],
                            start=(j == 0), stop=(j == 3), perf_mode=DR)
                    nc.vector.tensor_scalar(
                        kT[:, ft, ch * 512:ch * 512 + 512], ps, 1.0 / WS,
                        None, ALU.mult)

        wv_ap = Wv8.ap().rearrange("(dt dp) f -> dp dt f", dp=P)
        for c in range(2):
            wc = wpool.tile([P, 8, 512], F8, tag="w8", name=f"wv_{c}")
            nc.sync.dma_start(out=wc, in_=wv_ap[:, :, c * 512:c * 512 + 512])
            for tt in range(8):
                ps = pmm.tile([P, 512], F32, tag="mm", name=f"v_{c}_{tt}")
                for j in range(4):
                    nc.tensor.matmul(
                        ps, h8[:, 2 * j:2 * j + 2, tt * P:tt * P + P],
                        wc[:, 2 * j:2 * j + 2, :],
                        start=(j == 0), stop=(j == 3), perf_mode=DR)
                nc.vector.tensor_scalar(
                    vt[:, tt, c * 8:c * 8 + 8, 0:HD],
                    ps.rearrange("p (h d) -> p h d", h=8), 1.0 / WS,
                    None, ALU.mult)
        pclose("ph")

        # ctx applies become per-head DVE units inside self-attention so the
        # attention-critical DVE chain is never queued behind them.
        def mk_capply(c, dt):
            def f():
                cs = slice(c * 512, c * 512 + 512)
                A, Bt = ab_c[c]
                t1 = tmp.tile([P, 512], F16, tag="ap1")
                nc.vector.tensor_mul(t1, cs16[:, dt, cs], A)
                nc.vector.tensor_add(hc8[:, dt, cs], t1, Bt)
            return f

        dunits = [mk_capply(c, dt) for c in range(2) for dt in range(8)]

        # ---- k2/v2 as filler units (run inside ACT-bound attention) ------
        k2T = pcatt1.tile([P, 8, MCTX], F16, tag="k2T")
        v2t = pcatt1.tile([P, 8, NH, HD + 1], F8, tag="v2t")
        nc.vector.memset(v2t[:, :, :, HD:HD + 1], 1.0)
        wk2_ap = Wk28.ap().rearrange("(dt dp) f -> dp dt f", dp=P)
        wv2_ap = Wv28.ap().rearrange("(dt dp) f -> dp dt f", dp=P)
        wch = {}

        def mk_k2(c, fs, ch):
            def f():
                if fs == 0:
                    wc = wpool.tile([P, 8, 512], F8, tag="w8",
                                    name=f"wk2_{c}_{ch}")
                    nc.sync.dma_start(
                        out=wc, in_=wk2_ap[:, :, c * 512:c * 512 + 512])
                    wch["k", c] = wc
                wc = wch["k", c]
                ft = c * 4 + fs
                ps = pmm.tile([P, 512], F32, tag="mm", name=f"k2_{ft}_{ch}")
                for j in range(4):
                    nc.tensor.matmul(
                        ps, wc[:, 2 * j:2 * j + 2, fs * P:fs * P + P],
                        hc8[:, 2 * j:2 * j + 2, ch * 512:ch * 512 + 512],
                        start=(j == 0), stop=(j == 3), perf_mode=DR)
                nc.vector.tensor_scalar(
                    k2T[:, ft, ch * 512:ch * 512 + 512], ps, 1.0 / WS,
                    None, ALU.mult)
            return f

        def mk_v2(c, tt):
            def f():
                if tt % 4 == 0:
                    wc = wpool.tile([P, 8, 512], F8, tag="w8",
                                    name=f"wv2_{c}_{tt}")
                    nc.sync.dma_start(
                        out=wc, in_=wv2_ap[:, :, c * 512:c * 512 + 512])
                    wch["v", c] = wc
                wc = wch["v", c]
                ps = pmm.tile([P, 512], F32, tag="mm", name=f"v2_{c}_{tt}")
                for j in range(4):
                    nc.tensor.matmul(
                        ps, hc8[:, 2 * j:2 * j + 2, tt * P:tt * P + P],
                        wc[:, 2 * j:2 * j + 2, :],
                        start=(j == 0), stop=(j == 3), perf_mode=DR)
                nc.vector.tensor_scalar(
                    v2t[:, tt, c * 8:c * 8 + 8, 0:HD],
                    ps.rearrange("p (h d) -> p h d", h=8), 1.0 / WS,
                    None, ALU.mult)
            return f

        # hc8-chunk-0 consumers first (ready after ~4 heads of ctx-apply
        # dunits); heads 1-4 get no PE filler.
        units = ([(lambda: None)] * 4
                 + [mk_k2(c, fs, 0) for c in range(2) for fs in range(4)]
                 + [mk_v2(c, tt) for c in range(2) for tt in range(4)]
                 + [mk_k2(c, fs, 1) for c in range(2) for fs in range(4)]
                 + [mk_v2(c, tt) for c in range(2) for tt in range(4, 8)])

        # ---- self-attention + out-proj + residual ------------------------
        saT = psa.tile([P, 8, Q], F8, tag="saT")
        _attention(nc, pp, kT, vt, qT, saT, m01t, tb_t,
                   fillers=units[:16], dunits=dunits)

        xa16 = pxa.tile([P, 8, Q], F16, tag="xa16")
        for ft, ps in _proj8(nc, pp, Wso8, saT, "so"):
            nc.vector.scalar_tensor_tensor(xa16[:, ft, :], ps, 1.0 / WS,
                                           xs[:, ft, 0:Q], ALU.mult, ALU.add)
        pclose("psa")
        pclose("pc")
        pclose("pattn1")
        pclose("px")

        # ---- cross-attention ---------------------------------------------
        for f in units[16:24]:
            f()
        A_xa, B_xa = _stats(nc, pp, xa16, 0)
        for f in units[24:]:
            f()

        pq2 = popen("pq2", "left")
        q2T = pq2.tile([P, 8, Q], F16, tag="q2T")
        phq = popen("phq", "left")
        hq8 = phq.tile([P, 8, Q], F8, tag="hq8")
        _apply(nc, pp, xa16, 0, A_xa, B_xa, None, hq8)
        for ft, ps in _proj8(nc, pp, Wq28, hq8, "q2"):
            nc.vector.tensor_scalar(q2T[:, ft, :], ps, 1.0 / WSQ, None,
                                    ALU.mult)
        pclose("phq")

        pca = popen("pca", "right")
        caT = pca.tile([P, 8, Q], F8, tag="caT")
        _attention(nc, pp, k2T, v2t, q2T, caT, None, None)

        xb16 = pxb.tile([P, 8, Q], F16, tag="xb16")
        for ft, ps in _proj8(nc, pp, Wco8, caT, "co"):
            nc.vector.scalar_tensor_tensor(xb16[:, ft, :], ps, 1.0 / WS,
                                           xa16[:, ft, :], ALU.mult, ALU.add)
        pclose("pca")
        pclose("pq2")
        pclose("pcatt1")
        pclose("phc8")
        pclose("pxa")

        # ---- MLP ----------------------------------------------------------
        A_xb, B_xb = _stats(nc, pp, xb16, 0)
        pmlp = popen("pmlp", "left")
        h28 = pmlp.tile([P, 8, Q], F8, tag="h28")
        _apply(nc, pp, xb16, 0, A_xb, B_xb, None, h28)

        gt = pmlp.tile([P, 32, Q], F8, tag="gt")
        w1_ap = W18.ap().rearrange("(dt dp) f -> dp dt f", dp=P)
        for c in range(8):
            wc = wpool.tile([P, 8, 512], F8, tag="w8", name=f"w1_{c}")
            nc.sync.dma_start(out=wc, in_=w1_ap[:, :, c * 512:c * 512 + 512])
            for fs in range(4):
                ps = pmm.tile([P, 512], F32, tag="mm", name=f"f1_{c}_{fs}")
                for j in range(4):
                    nc.tensor.matmul(
                        ps, wc[:, 2 * j:2 * j + 2, fs * P:fs * P + P],
                        h28[:, 2 * j:2 * j + 2, :],
                        start=(j == 0), stop=(j == 3), perf_mode=DR)
                nc.scalar.activation(gt[:, c * 4 + fs, :], ps, AFT.Gelu,
                                     scale=1.0 / WS)

        ot = pmlp.tile([P, 8, Q], F32, tag="ot")
        w2_ap = W28.ap().rearrange("(dt dp) f -> dp dt f", dp=P)
        outT_r = outT.ap().rearrange("(dt dp) q -> dp dt q", dp=P)
        for ch in range(2):
            for half in range(2):
                pss = [pmm.tile([P, 512], F32, tag="mm",
                                name=f"f2_{ch}_{half}_{e}") for e in range(2)]
                for g in range(4):
                    wc = wpool.tile([P, 8, 512], F8, tag="w8",
                                    name=f"w2_{ch}_{half}_{g}")
                    nc.sync.dma_start(
                        out=wc, in_=w2_ap[:, g * 8:g * 8 + 8,
                                          ch * 512:ch * 512 + 512])
                    for e in range(2):
                        for j in range(4):
                            nc.tensor.matmul(
                                pss[e],
                                wc[:, 2 * j:2 * j + 2,
                                   (half * 2 + e) * P:(half * 2 + e + 1) * P],
                                gt[:, g * 8 + 2 * j:g * 8 + 2 * j + 2, :],
                                start=(g == 0 and j == 0),
                                stop=(g == 3 and j == 3), perf_mode=DR)
                for e in range(2):
                    ft = ch * 4 + half * 2 + e
                    nc.vector.scalar_tensor_tensor(
                        ot[:, ft, :], pss[e], 1.0 / WS2, xb16[:, ft, :],
                        ALU.mult, ALU.add)
                    nc.sync.dma_start(out=outT_r[:, ft, :], in_=ot[:, ft, :])
        pclose("pxb")
        pclose("pmlp")

    nc.compile()
    return nc


# ----------------------------------------------------------------------------
# host side
# ----------------------------------------------------------------------------

def _prep_inputs(x, context, sa_mask, W_qkv, W_self_out, W_q, W_kv, W_cross_out,
                 W_fc1, W_fc2, g_norm1, g_query_norm, g_context_norm, g_norm2):
    f32, f16 = np.float32, np.float16
    f8 = mybir.dt.np(F8)
    g1 = np.asarray(g_norm1, f32)[:, None]
    gq = np.asarray(g_query_norm, f32)[:, None]
    gc = np.asarray(g_context_norm, f32)[:, None]
    g2 = np.asarray(g_norm2, f32)[:, None]
    W_qkv = np.asarray(W_qkv, f32)
    W_kv = np.asarray(W_kv, f32)
    cw = lambda a: np.ascontiguousarray(a.astype(f16))
    cw8 = lambda a, s: np.ascontiguousarray((a * f32(s)).astype(f8))
    weights = {
        "Wq8": cw8(W_qkv[0:D].T * g1 * f32(SCALE), WSQ),
        "Wk8": cw8(W_qkv[D:2 * D].T * g1, WS),
        "Wv8": cw8(W_qkv[2 * D:3 * D].T * g1, WS),
        "Wso8": cw8(np.asarray(W_self_out, f32).T, WS),
        "Wq28": cw8(np.asarray(W_q, f32).T * gq * f32(SCALE), WSQ),
        "Wk28": cw8(W_kv[0:D].T * gc, WS),
        "Wv28": cw8(W_kv[D:2 * D].T * gc, WS),
        "Wco8": cw8(np.asarray(W_cross_out, f32).T, WS),
        "W18": cw8(np.asarray(W_fc1, f32).T * g2, WS),
        "W28": cw8(np.asarray(W_fc2, f32).T, WS2),
    }
    in_maps = []
    for c in range(8):
        b, s = c // 2, c % 2
        own = np.arange(s * Q, s * Q + Q)
        idx = np.concatenate([own, np.arange((1 - s) * Q, (1 - s) * Q + Q)])
        xb = np.asarray(x[b], f32)
        m01 = (np.asarray(sa_mask[b])[np.ix_(own, own)] != 0).astype(f16)
        m = dict(weights)
        m["x16"] = np.ascontiguousarray(xb[idx].T.astype(f16))
        m["m01"] = np.ascontiguousarray(m01.T)
        m["tbias"] = np.full((P, 1), NEG if s == 0 else 0.0, f32)
        m["ctx16"] = np.ascontiguousarray(
            np.asarray(context[b], f32).T.astype(f16))
        in_maps.append(m)
    return in_maps


def _check_mask(sa_mask):
    mask = np.asarray(sa_mask)
    lo, hi = np.arange(0, Q), np.arange(Q, L)
    for b in range(B):
        if not np.all(mask[b][np.ix_(lo, hi)] == 0):
            return False
        if not np.all(mask[b][np.ix_(hi, lo)] != 0):
            return False
    return True


def _gather(results, x_dtype):
    out = np.empty((B, L, D), np.float32)
    for c in range(8):
        b, s = c // 2, c % 2
        out[b, s * Q:(s + 1) * Q, :] = results[c]["outT"].T
    return out.astype(x_dtype, copy=False)


def _run(trace=False, **inputs):
    assert _check_mask(inputs["sa_mask"]), \
        "sa_mask does not have the expected causal block structure"
    if "nc" not in _CACHE:
        _CACHE["nc"] = build_program()
    nc = _CACHE["nc"]
    in_maps = _prep_inputs(**inputs)
    res = run_bass_kernel_spmd(nc, in_maps, list(range(8)), trace=trace)
    out = _gather(res.results, np.asarray(inputs["x"]).dtype)
    return out, res


def kernel(**inputs) -> np.ndarray:
    out, _ = _run(trace=False, **inputs)
    return out


def kernel_traced(**inputs):
    """Returns (output, exec_time_ns). Used by test.py."""
    import sys, types
    try:
        import antenv
        import trn_agent_boot.trn_boot as tb
        import concourse.bass_utils as bu
        if "antenv.axon_hooks" not in sys.modules:
            hook = tb._ntff_profile_via_ctypes('/opt/axon/libaxon_pjrt.so')
            mod = types.ModuleType("antenv.axon_hooks")
            mod.get_axon_ntff_profile_hook = lambda: hook
            mod.set_axon_ntff_profile_hook = lambda h: None
            sys.modules['antenv.axon_hooks'] = mod
            antenv.axon_hooks = mod
        bu.upload_artifacts = lambda tmpdir: "local://skipped"
    except Exception as e:
        print(f"ntff hook install failed: {e}")
    out, res = _run(trace=True, **inputs)
    return out, res.exec_time_ns
